# revision 1
# baseline (speedup 1.0000x reference)
"""Trainium2 Bass kernel for nn_BktModel: BKT HMM forward filter over
A*S=5120 tiled subsequences of length T=1024, followed by scatter into
per-ability timelines and a sequential-Bayesian ability average.

Strategy
--------
Device (8 cores, data-parallel over the A*S row axis, 640 rows/core):
  The sequential 2-state HMM filter is the only hard part. We run it
  chunk-parallel: each row's T=1024 steps split into C=32 chunks of
  CL=32 steps. The *unnormalized* filter is linear (alpha' = M_t alpha),
  so each chunk is propagated for two basis inits e0=[1,0], e1=[0,1]
  simultaneously across all (row, chunk) tasks -> fat (128,160) vector
  ops with only 32 sequential steps. A shared rescale (by basis A's
  state sum) every 8 steps prevents underflow without breaking
  linearity. Chunk-composite maps (the basis endpoints) are then chained
  sequentially (32 tiny ops) to get each chunk's true start state, and
  the per-step prediction prob p_t = (a0*g + a1*h)/(a0+a1) is
  reconstructed in bulk as separate numerator/denominator outputs.

Host (inside kernel()): parameter gathers / sigmoids (prologue), final
p = num/den + log, the trial_id scatter, and the Bayesian reduction
(epilogue) - executed with jax on CPU, mirroring the reference ops.
"""

import numpy as np

# Problem shape (hardcoded per contract)
B0, K, T, A = 128, 8, 1024, 5
N_KCS, N_PROBLEMS = 50, 1000
MAX_LEN = K * T
S = B0 * K            # 1024 subsequences
AS = A * S            # 5120 rows after tiling across ability levels
EPS = 1e-12

NCORES = 8
RPC = AS // NCORES    # 640 rows per core
J = RPC // 128        # 5 row-blocks of 128 partitions
C = 32                # chunks per row
CL = T // C           # 32 steps per chunk
TASKS = C * J         # 160 (chunk, row-block) tasks per partition
NFLAT = CL * TASKS    # 5120 columns in time-major layout
REN = 16              # rescale period (steps)

LAST_EXEC_NS = None


def _pack(full):
    """(640, T) -> (128, NFLAT) with [p, t*TASKS + c*J + j] = full[j*128+p, c*CL+t]."""
    return np.ascontiguousarray(
        full.reshape(J, 128, C, CL).transpose(1, 3, 2, 0).reshape(128, NFLAT)
    )


def _unpack(packed):
    """Inverse of _pack."""
    return packed.reshape(128, CL, C, J).transpose(3, 0, 2, 1).reshape(RPC, T)


def _pack_row(val):
    """(640,) per-row values -> (128, TASKS) broadcast across chunks."""
    v2 = val.reshape(J, 128).T                      # (128, J)
    return np.ascontiguousarray(
        np.broadcast_to(v2[:, None, :], (128, C, J)).reshape(128, TASKS)
    )


def _pack_init(val):
    """(640,) -> (128, J)."""
    return np.ascontiguousarray(val.reshape(J, 128).T)


def _build_nc():
    import concourse.bass as bass
    import concourse.tile as tile
    from concourse import mybir
    from contextlib import ExitStack

    f32 = mybir.dt.float32
    nc = bass.Bass()

    # one input tensor: L0 | L1 | W00 | W10 | W01 | W11 | AI0 | AI1 | AIB0 | AIB1
    NIN = 2 * NFLAT + 6 * TASKS + 2 * J
    dIN = nc.declare_dram_parameter("IN", [128, NIN], f32, isOutput=False)
    # one output tensor: AL0 | AL1
    dOUT = nc.declare_dram_parameter("OUT", [128, 2 * NFLAT], f32, isOutput=True)

    with ExitStack() as ctx:
        tc = ctx.enter_context(tile.TileContext(nc))
        const = ctx.enter_context(tc.tile_pool(name="const", bufs=1))
        big = ctx.enter_context(tc.tile_pool(name="big", bufs=1))
        work = ctx.enter_context(tc.tile_pool(name="work", bufs=2))
        chain = ctx.enter_context(tc.tile_pool(name="chain", bufs=2))

        V = nc.vector

        # Single-input DVE "touch" after each DMA load: absorbs the DMA-queue
        # semaphore wait so no downstream TensorTensor needs >1 sync wait
        # (this codegen allows one wait slot per TT instruction).
        touch_n = [0]

        def touch(tl):
            tt = const.tile([128, 1], f32, tag=f"touch{touch_n[0]}")
            touch_n[0] += 1
            V.tensor_copy(tt[:], tl[:, 0:1])

        tin = big.tile([128, NIN], f32, tag="tin")
        PRE = 8 * TASKS   # first 8 scan steps' worth of L0/L1
        for lo, hi in ((2 * NFLAT, NIN),              # W + AI (tiny, first)
                       (0, PRE),                      # L0 prefix
                       (NFLAT, NFLAT + PRE),          # L1 prefix
                       (PRE, NFLAT),                  # L0 tail
                       (NFLAT + PRE, 2 * NFLAT)):     # L1 tail
            nc.sync.dma_start(out=tin[:, lo:hi], in_=dIN[:, lo:hi])
            tt = const.tile([128, 1], f32, tag=f"touchd{lo}")
            V.tensor_copy(tt[:], tin[:, lo:lo + 1])
        tL0 = tin[:, 0:NFLAT]
        tL1 = tin[:, NFLAT:2 * NFLAT]
        base = 2 * NFLAT
        w00 = tin[:, base + 0 * TASKS:base + 1 * TASKS]
        w10 = tin[:, base + 1 * TASKS:base + 2 * TASKS]
        w01 = tin[:, base + 2 * TASKS:base + 3 * TASKS]
        w11 = tin[:, base + 3 * TASKS:base + 4 * TASKS]
        ai0 = tin[:, base + 4 * TASKS:base + 4 * TASKS + J]
        ai1 = tin[:, base + 4 * TASKS + J:base + 4 * TASKS + 2 * J]
        aib_base = base + 4 * TASKS + 2 * J
        aib0 = tin[:, aib_base:aib_base + TASKS]
        aib1 = tin[:, aib_base + TASKS:aib_base + 2 * TASKS]

        # basis trajectories: CL+1 state columns (col t = state before step t)
        a0A = big.tile([128, (CL + 1) * TASKS], f32, tag="a0A")
        a1A = big.tile([128, (CL + 1) * TASKS], f32, tag="a1A")
        a0B = big.tile([128, (CL + 1) * TASKS], f32, tag="a0B")
        a1B = big.tile([128, (CL + 1) * TASKS], f32, tag="a1B")
        V.memset(a0A[:, 0:TASKS], 1.0)
        V.memset(a1A[:, 0:TASKS], 0.0)
        V.memset(a0B[:, 0:TASKS], 0.0)
        V.memset(a1B[:, 0:TASKS], 1.0)

        for t in range(CL):
            cur = slice(t * TASKS, (t + 1) * TASKS)
            nxt = slice((t + 1) * TASKS, (t + 2) * TASKS)
            for x0, x1 in ((a0A, a1A), (a0B, a1B)):
                b0 = work.tile([128, TASKS], f32, tag="b0")
                b1 = work.tile([128, TASKS], f32, tag="b1")
                V.tensor_mul(b0[:], x0[:, cur], tin[:, t * TASKS:(t + 1) * TASKS])
                V.tensor_mul(b1[:], x1[:, cur],
                             tin[:, NFLAT + t * TASKS:NFLAT + (t + 1) * TASKS])
                m0 = work.tile([128, TASKS], f32, tag="m0")
                m1 = work.tile([128, TASKS], f32, tag="m1")
                V.tensor_mul(m0[:], b0[:], w00)
                V.tensor_mul(m1[:], b1[:], w10)
                V.tensor_add(x0[:, nxt], m0[:], m1[:])
                m2 = work.tile([128, TASKS], f32, tag="m2")
                m3 = work.tile([128, TASKS], f32, tag="m3")
                V.tensor_mul(m2[:], b0[:], w01)
                V.tensor_mul(m3[:], b1[:], w11)
                V.tensor_add(x1[:, nxt], m2[:], m3[:])
            if (t + 1) % REN == 0:
                s = work.tile([128, TASKS], f32, tag="s")
                iv = work.tile([128, TASKS], f32, tag="iv")
                V.tensor_add(s[:], a0A[:, nxt], a1A[:, nxt])
                V.reciprocal(iv[:], s[:])
                for buf in (a0A, a1A, a0B, a1B):
                    V.tensor_mul(buf[:, nxt], buf[:, nxt], iv[:])

        # prefix-compose the chunk maps by log-doubling, then apply to the
        # initial state to get each chunk's start coeffs (a0t, a1t)
        a0t = const.tile([128, TASKS], f32, tag="a0t")
        a1t = const.tile([128, TASKS], f32, tag="a1t")
        E = slice(CL * TASKS, (CL + 1) * TASKS)   # endpoint maps, task layout
        pc0 = const.tile([128, TASKS], f32, tag="pc0")
        pc1 = const.tile([128, TASKS], f32, tag="pc1")
        pc2 = const.tile([128, TASKS], f32, tag="pc2")
        pc3 = const.tile([128, TASKS], f32, tag="pc3")
        pn0 = const.tile([128, TASKS], f32, tag="pn0")
        pn1 = const.tile([128, TASKS], f32, tag="pn1")
        pn2 = const.tile([128, TASKS], f32, tag="pn2")
        pn3 = const.tile([128, TASKS], f32, tag="pn3")
        pcur = [pc0, pc1, pc2, pc3]
        pnx = [pn0, pn1, pn2, pn3]
        # P = [[p00,p01],[p10,p11]] = [[a0A,a0B],[a1A,a1B]] at endpoints
        V.tensor_copy(pcur[0][:], a0A[:, E])
        V.tensor_copy(pcur[1][:], a0B[:, E])
        V.tensor_copy(pcur[2][:], a1A[:, E])
        V.tensor_copy(pcur[3][:], a1B[:, E])
        sft = 1
        while sft < C:
            s = sft * J
            # head: unchanged
            for i in range(4):
                V.tensor_copy(pnx[i][:, 0:s], pcur[i][:, 0:s])
            # tail: P'[c] = P[c] @ P[c - sft]
            A00 = pcur[0][:, s:TASKS]; A01 = pcur[1][:, s:TASKS]
            A10 = pcur[2][:, s:TASKS]; A11 = pcur[3][:, s:TASKS]
            B00 = pcur[0][:, 0:TASKS - s]; B01 = pcur[1][:, 0:TASKS - s]
            B10 = pcur[2][:, 0:TASKS - s]; B11 = pcur[3][:, 0:TASKS - s]
            for i, (ax, ay, bx, by) in enumerate((
                    (A00, A01, B00, B10),   # C00 = A00*B00 + A01*B10
                    (A00, A01, B01, B11),   # C01 = A00*B01 + A01*B11
                    (A10, A11, B00, B10),   # C10
                    (A10, A11, B01, B11))):  # C11
                u = work.tile([128, TASKS], f32, tag="m0")
                v = work.tile([128, TASKS], f32, tag="m1")
                V.tensor_mul(u[:, 0:TASKS - s], ax, bx)
                V.tensor_mul(v[:, 0:TASKS - s], ay, by)
                V.tensor_add(pnx[i][:, s:TASKS], u[:, 0:TASKS - s],
                             v[:, 0:TASKS - s])
            # rescale columns by 1/(C00+C10) to keep entries in f32 range
            sa = work.tile([128, TASKS], f32, tag="s")
            iva = work.tile([128, TASKS], f32, tag="iv")
            V.tensor_add(sa[:], pnx[0][:], pnx[2][:])
            V.reciprocal(iva[:], sa[:])
            for i in range(4):
                V.tensor_mul(pnx[i][:], pnx[i][:], iva[:])
            pcur, pnx = pnx, pcur
            sft *= 2
        # App[c] = P[c] @ ainit ; a0t[c] = App[c-1] (exclusive), a0t[0] = ainit
        ap0 = work.tile([128, TASKS], f32, tag="m2")
        ap1 = work.tile([128, TASKS], f32, tag="m3")
        u0 = work.tile([128, TASKS], f32, tag="m0")
        v0 = work.tile([128, TASKS], f32, tag="m1")
        V.tensor_mul(u0[:], pcur[0][:], aib0)
        V.tensor_mul(v0[:], pcur[1][:], aib1)
        V.tensor_add(ap0[:], u0[:], v0[:])
        u1 = work.tile([128, TASKS], f32, tag="m0")
        v1 = work.tile([128, TASKS], f32, tag="m1")
        V.tensor_mul(u1[:], pcur[2][:], aib0)
        V.tensor_mul(v1[:], pcur[3][:], aib1)
        V.tensor_add(ap1[:], u1[:], v1[:])
        V.tensor_copy(a0t[:, 0:J], ai0)
        V.tensor_copy(a1t[:, 0:J], ai1)
        V.tensor_copy(a0t[:, J:TASKS], ap0[:, 0:TASKS - J])
        V.tensor_copy(a1t[:, J:TASKS], ap1[:, 0:TASKS - J])

        # bulk reconstruction of the filter state at every step; the final
        # p = (al0*g + al1*h)/(al0+al1) happens on the host
        tout = big.tile([128, 2 * NFLAT], f32, tag="tout")
        for t in range(CL):
            sl = slice(t * TASKS, (t + 1) * TASKS)
            sl1 = slice(NFLAT + t * TASKS, NFLAT + (t + 1) * TASKS)
            x0 = work.tile([128, TASKS], f32, tag="m0")
            y0 = work.tile([128, TASKS], f32, tag="m1")
            V.tensor_mul(x0[:], a0t[:], a0A[:, sl])
            V.tensor_mul(y0[:], a1t[:], a0B[:, sl])
            V.tensor_add(tout[:, sl], x0[:], y0[:])
            x1 = work.tile([128, TASKS], f32, tag="m2")
            y1 = work.tile([128, TASKS], f32, tag="m3")
            V.tensor_mul(x1[:], a0t[:], a1A[:, sl])
            V.tensor_mul(y1[:], a1t[:], a1B[:, sl])
            V.tensor_add(tout[:, sl1], x1[:], y1[:])

        QT = CL // 4
        for q in range(4):
            qs0 = slice(q * QT * TASKS, (q + 1) * QT * TASKS)
            qs1 = slice(NFLAT + q * QT * TASKS, NFLAT + (q + 1) * QT * TASKS)
            nc.sync.dma_start(out=dOUT[:, qs0], in_=tout[:, qs0])
            nc.sync.dma_start(out=dOUT[:, qs1], in_=tout[:, qs1])

    _split_multi_waits(nc, mybir)
    return nc


def _split_multi_waits(nc, mybir):
    """This neuronx-cc codegen allows only one sync-wait slot per
    instruction; hoist all but the last wait of any multi-wait instruction
    onto single-wait NoOps inserted just before it (same engine, same
    block) - sequential waits are semantically identical to ANDed waits."""
    k = 0
    for f in nc.m.functions:
        for b in f.blocks:
            new_list = []
            for inst in b.instructions:
                si = inst.sync_info
                if si is not None and si.on_wait and len(si.on_wait) > 1:
                    waits = list(si.on_wait)
                    for w in waits[:-1]:
                        nop = mybir.InstNoOp(
                            name=f"I-wsplit-{k}",
                            sync_info=mybir.SyncInfo(on_wait=[w], on_update=[]),
                            engine=inst.engine,
                        )
                        k += 1
                        new_list.append(nop)
                    inst.sync_info = mybir.SyncInfo(
                        on_wait=[waits[-1]], on_update=list(si.on_update))
                new_list.append(inst)
            if k:
                b.instructions[:] = new_list


def kernel(dynamics_logits, obs_logits_kc, obs_logits_problem, ability_levels,
           padded_trial_id, padded_problem, padded_correct, kc, ytrue):
    global LAST_EXEC_NS
    import os
    import jax
    import jax.numpy as jnp

    cpu = jax.devices("cpu")[0]

    dyn_l = np.asarray(dynamics_logits, np.float32)
    obs_kc = np.asarray(obs_logits_kc, np.float32)
    obs_pr = np.asarray(obs_logits_problem, np.float32)
    abil = np.asarray(ability_levels, np.float32)
    tid = np.asarray(padded_trial_id, np.int32)
    prob = np.asarray(padded_problem, np.int32)
    corr = np.asarray(padded_correct, np.int32)
    kc_a = np.asarray(kc, np.int32)
    yt = np.asarray(ytrue, np.int32)

    # ---- host prologue (mirrors reference lines, jax on CPU) ----
    with jax.default_device(cpu):
        ability = jnp.repeat(jnp.asarray(abil), S)            # (AS,)
        corr_t = jnp.tile(jnp.asarray(corr), (A, 1))          # (AS,T)
        prob_t = jnp.tile(jnp.asarray(prob), (A, 1))
        kc_t = jnp.tile(jnp.asarray(kc_a), (A,))
        dyn = jnp.asarray(dyn_l)[kc_t]                        # (AS,3)
        obs = jnp.asarray(obs_kc)[kc_t][:, None, :] + jnp.asarray(obs_pr)[prob_t]
        pG = jax.nn.sigmoid(obs[..., 0] + ability[:, None])   # (AS,T)
        pS = jax.nn.sigmoid(obs[..., 1] - ability[:, None])
        pL = jax.nn.sigmoid(dyn[:, 0])
        pF = jax.nn.sigmoid(dyn[:, 1])
        pI = jax.nn.sigmoid(dyn[:, 2])
        g = np.asarray(pG)
        h = np.asarray(1.0 - pS)                               # pc1
        yf = np.asarray(corr_t) == 1
        L0 = np.where(yf, g, 1.0 - g).astype(np.float32)       # p(y | not known)
        L1 = np.where(yf, h, 1.0 - h).astype(np.float32)       # p(y | known)
        pLn = np.asarray(pL); pFn = np.asarray(pF); pIn = np.asarray(pI)

    w00 = (1.0 - pLn).astype(np.float32)
    w10 = pFn.astype(np.float32)
    w01 = pLn.astype(np.float32)
    w11 = (1.0 - pFn).astype(np.float32)
    ai0 = (1.0 - pIn).astype(np.float32)
    ai1 = pIn.astype(np.float32)

    # ---- shard + pack per core ----
    in_maps = []
    for m in range(NCORES):
        r0, r1 = m * RPC, (m + 1) * RPC
        wai = np.concatenate([
            _pack_row(w00[r0:r1]),
            _pack_row(w10[r0:r1]),
            _pack_row(w01[r0:r1]),
            _pack_row(w11[r0:r1]),
            _pack_init(ai0[r0:r1]),
            _pack_init(ai1[r0:r1]),
            _pack_row(ai0[r0:r1]),
            _pack_row(ai1[r0:r1]),
        ], axis=1)
        in_maps.append({
            "IN": np.ascontiguousarray(np.concatenate(
                [_pack(L0[r0:r1]), _pack(L1[r0:r1]), wai], axis=1)),
        })

    # ---- build + run the Bass kernel on 8 cores ----
    from concourse.bass_utils import run_bass_kernel_spmd
    nc = _build_nc()
    import time as _time
    _t0 = _time.perf_counter()
    res = run_bass_kernel_spmd(nc, in_maps, list(range(NCORES)))
    LAST_EXEC_NS = (_time.perf_counter() - _t0) * 1e9

    # ---- unshard ----
    al0 = np.empty((AS, T), np.float32)
    al1 = np.empty((AS, T), np.float32)
    for m in range(NCORES):
        r0, r1 = m * RPC, (m + 1) * RPC
        outm = np.asarray(res.results[m]["OUT"])
        al0[r0:r1] = _unpack(outm[:, :NFLAT])
        al1[r0:r1] = _unpack(outm[:, NFLAT:])

    # p_t = (al0*g + al1*h) / (al0+al1)  (scale-invariant in the alphas)
    p = (al0 * g + al1 * h) / (al0 + al1)

    # ---- host epilogue (mirrors reference lines, jax on CPU) ----
    with jax.default_device(cpu):
        pj = jnp.asarray(p)
        logprob_pred = jnp.log(jnp.clip(
            jnp.stack([1.0 - pj, pj], axis=-1), EPS))          # (AS,T,2)
        abil_ix = jnp.repeat(jnp.arange(A), S)
        tid_t = jnp.tile(jnp.asarray(tid), (A, 1))
        adj = tid_t + abil_ix[:, None] * (B0 * MAX_LEN)
        adj = jnp.where(tid_t == -1, -1, adj).reshape(-1)
        n_flat = A * B0 * MAX_LEN
        idx = jnp.where(adj > -1, adj, n_flat)
        buf = jnp.zeros((n_flat, 2), dtype=logprob_pred.dtype)
        buf = buf.at[idx].set(logprob_pred.reshape(-1, 2), mode="drop")
        result = jnp.transpose(buf.reshape(A, B0, MAX_LEN, 2), (1, 0, 2, 3))

        ytj = jnp.asarray(yt)
        mask = ytj > -1
        yc = jnp.where(mask, ytj, 0)
        obs_ll = jnp.take_along_axis(
            result, yc[:, None, :, None].astype(jnp.int32), axis=3)[..., 0]
        obs_ll = obs_ll * mask[:, None, :]
        prefix = jnp.cumsum(obs_ll, axis=2) - obs_ll
        from jax.scipy.special import logsumexp
        logw = prefix - logsumexp(prefix, axis=1, keepdims=True)
        logpred = logsumexp(result + logw[..., None], axis=1)
        out = np.asarray(logpred, dtype=np.float32)

    return out



# revision 4
# speedup vs baseline: 4.1987x; 4.1987x over previous
"""Trainium2 Bass kernel for nn_BktModel: BKT HMM forward filter over
A*S=5120 tiled subsequences of length T=1024, followed by scatter into
per-ability timelines and a sequential-Bayesian ability average.

Strategy
--------
The spmd call is transfer-bound (axon tunnel ~30-45 MB/s), so the kernel
minimizes bytes on the wire:

Inputs per core (128 subsequences, all 5 ability levels expanded on
device): three f16 (128,1024) arrays U = w*c0, V = -w*c1, W = w, where
c0/c1 are the per-(s,t) observation logits and w = 2*correct-1, plus a
(128,16) f32 block of per-subsequence dynamics probabilities and the 5
ability levels. The device computes the likelihoods
  L0 = sigmoid(U + A_j*W), L1 = sigmoid(V + A_j*W)
for each ability j via the scalar engine.

The sequential 2-state HMM filter runs chunk-parallel: each row's T=1024
steps split into C=32 chunks of CL=32 steps, propagated for two basis
inits simultaneously across all (chunk, ability) tasks -> (128,160)
vector ops with only 32 sequential steps (the unnormalized filter is
linear). A shared rescale every 16 steps prevents underflow. Chunk maps
are prefix-composed by log-doubling, the true per-step states are
reconstructed in bulk, and the device emits the odds ratio
  r = p/(1-p),  p = (a0*g + a1*h)/(a0+a1)
as a single f16 (128,5120) output per core.

Host (outside the timed spmd call): parameter gathers, packing, the
log-odds -> log-prob conversion, the trial_id scatter, and the Bayesian
reduction (mirroring the reference ops on CPU).
"""

import numpy as np

# Problem shape (hardcoded per contract)
B0, K, T, A = 128, 8, 1024, 5
N_KCS, N_PROBLEMS = 50, 1000
MAX_LEN = K * T
S = B0 * K            # 1024 subsequences
AS = A * S            # 5120 rows after tiling across ability levels
EPS = 1e-12

NCORES = 8
SPC = S // NCORES     # 128 subsequences per core (= partition dim)
J = A                 # ability levels, inner task axis
C = 32                # chunks per row
CL = T // C           # 32 steps per chunk
TASKS = C * J         # 160 (chunk, ability) tasks per partition
NFLAT = CL * TASKS    # 5120 columns in time-major layout
REN = 16              # rescale period (steps)

NF16 = 3 * T          # U | V | W
NF32 = 16             # w00 w10 w01 w11 ai0 ai1 A0..A4 pad

LAST_EXEC_NS = None
_NC = None


def _build_nc():
    import concourse.bass as bass
    import concourse.tile as tile
    from concourse import mybir
    from contextlib import ExitStack

    f32 = mybir.dt.float32
    f16 = mybir.dt.float16
    OP = mybir.AluOpType
    Sig = mybir.ActivationFunctionType.Sigmoid
    nc = bass.Bass()

    dF16 = nc.declare_dram_parameter("F16", [128, NF16], f16, isOutput=False)
    dF32 = nc.declare_dram_parameter("F32", [128, NF32], f32, isOutput=False)
    dOUT = nc.declare_dram_parameter("OUT", [128, NFLAT], f16, isOutput=True)

    with ExitStack() as ctx:
        tc = ctx.enter_context(tile.TileContext(nc))
        const = ctx.enter_context(tc.tile_pool(name="const", bufs=1))
        big = ctx.enter_context(tc.tile_pool(name="big", bufs=1))
        work = ctx.enter_context(tc.tile_pool(name="work", bufs=2))
        chain = ctx.enter_context(tc.tile_pool(name="chain", bufs=2))

        V = nc.vector
        SC = nc.scalar

        # Single-input DVE "touch" after each DMA load: absorbs the DMA-queue
        # semaphore wait so no downstream op needs >1 sync wait.
        touch_n = [0]

        def touch(tl):
            tt = const.tile([128, 1], f32, tag=f"touch{touch_n[0]}")
            touch_n[0] += 1
            V.tensor_copy(tt[:], tl[:, 0:1])

        tsc = const.tile([128, NF32], f32, tag="tsc")
        nc.sync.dma_start(out=tsc[:], in_=dF32[:])
        touch(tsc)
        tf16 = big.tile([128, NF16], f16, tag="tf16")
        nc.sync.dma_start(out=tf16[:], in_=dF16[:])
        touch(tf16)

        w00s = tsc[:, 0:1]
        w10s = tsc[:, 1:2]
        w01s = tsc[:, 2:3]
        w11s = tsc[:, 3:4]
        ai0s = tsc[:, 4:5]
        ai1s = tsc[:, 5:6]

        Uf = big.tile([128, T], f32, tag="Uf")
        Vf = big.tile([128, T], f32, tag="Vf")
        Wf = big.tile([128, T], f32, tag="Wf")
        V.tensor_copy(Uf[:], tf16[:, 0:T])
        V.tensor_copy(Vf[:], tf16[:, T:2 * T])
        V.tensor_copy(Wf[:], tf16[:, 2 * T:3 * T])
        c0w = big.tile([128, T], f32, tag="c0w")  # 0.5*(1 - w)
        V.tensor_scalar(c0w[:], Wf[:], -0.5, 0.5, op0=OP.mult, op1=OP.add)

        # likelihoods per ability level, scan layout [t*TASKS + c*J + j]
        L0 = big.tile([128, NFLAT], f32, tag="L0")
        L1 = big.tile([128, NFLAT], f32, tag="L1")
        for j in range(J):
            Ajs = tsc[:, 6 + j:7 + j]
            argA = work.tile([128, T], f32, tag="argA")
            V.scalar_tensor_tensor(argA[:], Wf[:], Ajs, Uf[:],
                                   op0=OP.mult, op1=OP.add)
            SC.activation(
                L0[:].rearrange("p (t c j) -> p t c j", c=C, j=J)[:, :, :, j],
                argA[:].rearrange("p (t c) -> p t c", c=C), Sig)
            argB = work.tile([128, T], f32, tag="argB")
            V.scalar_tensor_tensor(argB[:], Wf[:], Ajs, Vf[:],
                                   op0=OP.mult, op1=OP.add)
            SC.activation(
                L1[:].rearrange("p (t c j) -> p t c j", c=C, j=J)[:, :, :, j],
                argB[:].rearrange("p (t c) -> p t c", c=C), Sig)

        # basis trajectories: CL+1 state columns (col t = state before step t)
        a0A = big.tile([128, (CL + 1) * TASKS], f32, tag="a0A")
        a1A = big.tile([128, (CL + 1) * TASKS], f32, tag="a1A")
        a0B = big.tile([128, (CL + 1) * TASKS], f32, tag="a0B")
        a1B = big.tile([128, (CL + 1) * TASKS], f32, tag="a1B")
        V.memset(a0A[:, 0:TASKS], 1.0)
        V.memset(a1A[:, 0:TASKS], 0.0)
        V.memset(a0B[:, 0:TASKS], 0.0)
        V.memset(a1B[:, 0:TASKS], 1.0)

        for t in range(CL):
            cur = slice(t * TASKS, (t + 1) * TASKS)
            nxt = slice((t + 1) * TASKS, (t + 2) * TASKS)
            l0t = L0[:, t * TASKS:(t + 1) * TASKS]
            l1t = L1[:, t * TASKS:(t + 1) * TASKS]
            for x0, x1 in ((a0A, a1A), (a0B, a1B)):
                b0 = work.tile([128, TASKS], f32, tag="b0")
                b1 = work.tile([128, TASKS], f32, tag="b1")
                V.tensor_mul(b0[:], x0[:, cur], l0t)
                V.tensor_mul(b1[:], x1[:, cur], l1t)
                tm0 = work.tile([128, TASKS], f32, tag="tm0")
                V.tensor_scalar_mul(tm0[:], b1[:], w10s)
                V.scalar_tensor_tensor(x0[:, nxt], b0[:], w00s, tm0[:],
                                       op0=OP.mult, op1=OP.add)
                tm1 = work.tile([128, TASKS], f32, tag="tm1")
                V.tensor_scalar_mul(tm1[:], b1[:], w11s)
                V.scalar_tensor_tensor(x1[:, nxt], b0[:], w01s, tm1[:],
                                       op0=OP.mult, op1=OP.add)
            if (t + 1) % REN == 0:
                s = work.tile([128, TASKS], f32, tag="s")
                iv = work.tile([128, TASKS], f32, tag="iv")
                V.tensor_add(s[:], a0A[:, nxt], a1A[:, nxt])
                V.reciprocal(iv[:], s[:])
                for buf in (a0A, a1A, a0B, a1B):
                    V.tensor_mul(buf[:, nxt], buf[:, nxt], iv[:])

        # prefix-compose the chunk maps by log-doubling, then apply to the
        # initial state to get each chunk's start coeffs (a0t, a1t)
        a0t = const.tile([128, TASKS], f32, tag="a0t")
        a1t = const.tile([128, TASKS], f32, tag="a1t")
        E = slice(CL * TASKS, (CL + 1) * TASKS)   # endpoint maps, task layout
        pcur = [const.tile([128, TASKS], f32, name=f"pc{i}", tag=f"pc{i}")
                for i in range(4)]
        pnx = [const.tile([128, TASKS], f32, name=f"pn{i}", tag=f"pn{i}")
               for i in range(4)]
        # P = [[p00,p01],[p10,p11]] = [[a0A,a0B],[a1A,a1B]] at endpoints
        V.tensor_copy(pcur[0][:], a0A[:, E])
        V.tensor_copy(pcur[1][:], a0B[:, E])
        V.tensor_copy(pcur[2][:], a1A[:, E])
        V.tensor_copy(pcur[3][:], a1B[:, E])
        sft = 1
        while sft < C:
            s = sft * J
            # head: unchanged
            for i in range(4):
                V.tensor_copy(pnx[i][:, 0:s], pcur[i][:, 0:s])
            # tail: P'[c] = P[c] @ P[c - sft]
            A00 = pcur[0][:, s:TASKS]; A01 = pcur[1][:, s:TASKS]
            A10 = pcur[2][:, s:TASKS]; A11 = pcur[3][:, s:TASKS]
            B00 = pcur[0][:, 0:TASKS - s]; B01 = pcur[1][:, 0:TASKS - s]
            B10 = pcur[2][:, 0:TASKS - s]; B11 = pcur[3][:, 0:TASKS - s]
            for i, (ax, ay, bx, by) in enumerate((
                    (A00, A01, B00, B10),   # C00 = A00*B00 + A01*B10
                    (A00, A01, B01, B11),   # C01 = A00*B01 + A01*B11
                    (A10, A11, B00, B10),   # C10
                    (A10, A11, B01, B11))):  # C11
                u = chain.tile([128, TASKS], f32, tag="m0")
                v = chain.tile([128, TASKS], f32, tag="m1")
                V.tensor_mul(u[:, 0:TASKS - s], ax, bx)
                V.tensor_mul(v[:, 0:TASKS - s], ay, by)
                V.tensor_add(pnx[i][:, s:TASKS], u[:, 0:TASKS - s],
                             v[:, 0:TASKS - s])
            # rescale columns by 1/(C00+C10) to keep entries in f32 range
            sa = chain.tile([128, TASKS], f32, tag="s")
            iva = chain.tile([128, TASKS], f32, tag="iv")
            V.tensor_add(sa[:], pnx[0][:], pnx[2][:])
            V.reciprocal(iva[:], sa[:])
            for i in range(4):
                V.tensor_mul(pnx[i][:], pnx[i][:], iva[:])
            pcur, pnx = pnx, pcur
            sft *= 2
        # App[c] = P[c] @ ainit ; a0t[c] = App[c-1] (exclusive), a0t[0] = ainit
        ap0 = chain.tile([128, TASKS], f32, tag="m2")
        ap1 = chain.tile([128, TASKS], f32, tag="m3")
        v0 = chain.tile([128, TASKS], f32, tag="m0")
        V.tensor_scalar_mul(v0[:], pcur[1][:], ai1s)
        V.scalar_tensor_tensor(ap0[:], pcur[0][:], ai0s, v0[:],
                               op0=OP.mult, op1=OP.add)
        v1 = chain.tile([128, TASKS], f32, tag="m1")
        V.tensor_scalar_mul(v1[:], pcur[3][:], ai1s)
        V.scalar_tensor_tensor(ap1[:], pcur[2][:], ai0s, v1[:],
                               op0=OP.mult, op1=OP.add)
        V.memset(a0t[:, 0:J], 1.0)
        V.tensor_scalar_mul(a0t[:, 0:J], a0t[:, 0:J], ai0s)
        V.memset(a1t[:, 0:J], 1.0)
        V.tensor_scalar_mul(a1t[:, 0:J], a1t[:, 0:J], ai1s)
        V.tensor_copy(a0t[:, J:TASKS], ap0[:, 0:TASKS - J])
        V.tensor_copy(a1t[:, J:TASKS], ap1[:, 0:TASKS - J])

        # bulk reconstruction + odds ratio r = num1/num0 in f16
        rout = big.tile([128, NFLAT], f16, tag="rout")
        for t in range(CL):
            sl = slice(t * TASKS, (t + 1) * TASKS)
            al0 = work.tile([128, TASKS], f32, tag="al0")
            x0 = work.tile([128, TASKS], f32, tag="x0")
            V.tensor_mul(al0[:], a0t[:], a0A[:, sl])
            V.tensor_mul(x0[:], a1t[:], a0B[:, sl])
            V.tensor_add(al0[:], al0[:], x0[:])
            al1 = work.tile([128, TASKS], f32, tag="al1")
            x1 = work.tile([128, TASKS], f32, tag="x1")
            V.tensor_mul(al1[:], a0t[:], a1A[:, sl])
            V.tensor_mul(x1[:], a1t[:], a1B[:, sl])
            V.tensor_add(al1[:], al1[:], x1[:])
            q1 = work.tile([128, TASKS], f32, tag="q1")
            q1b = work.tile([128, TASKS], f32, tag="q1b")
            V.tensor_mul(q1[:], al0[:], L0[:, sl])
            V.tensor_mul(q1b[:], al1[:], L1[:, sl])
            V.tensor_add(q1[:], q1[:], q1b[:])
            den = work.tile([128, TASKS], f32, tag="den")
            V.tensor_add(den[:], al0[:], al1[:])
            # num1 = c0w*den + w*q1  (per-chunk scalars broadcast over j)
            wt = Wf[:, t * C:(t + 1) * C].unsqueeze(2).broadcast_to([128, C, J])
            ct = c0w[:, t * C:(t + 1) * C].unsqueeze(2).broadcast_to([128, C, J])
            num1 = work.tile([128, TASKS], f32, tag="num1")
            qw = work.tile([128, TASKS], f32, tag="qw")
            V.tensor_mul(num1[:].rearrange("p (c j) -> p c j", j=J),
                         den[:].rearrange("p (c j) -> p c j", j=J), ct)
            V.tensor_mul(qw[:].rearrange("p (c j) -> p c j", j=J),
                         q1[:].rearrange("p (c j) -> p c j", j=J), wt)
            V.tensor_add(num1[:], num1[:], qw[:])
            num0 = work.tile([128, TASKS], f32, tag="num0")
            V.tensor_sub(num0[:], den[:], num1[:])
            rn = work.tile([128, TASKS], f32, tag="rn")
            V.reciprocal(rn[:], num0[:])
            V.tensor_mul(rout[:, sl], num1[:], rn[:])

        QT = CL // 4
        for q in range(4):
            qs = slice(q * QT * TASKS, (q + 1) * QT * TASKS)
            nc.sync.dma_start(out=dOUT[:, qs], in_=rout[:, qs])

    _split_multi_waits(nc, mybir)
    return nc


def _split_multi_waits(nc, mybir):
    """This neuronx-cc codegen allows only one sync-wait slot per
    instruction; hoist all but the last wait of any multi-wait instruction
    onto single-wait NoOps inserted just before it (same engine, same
    block) - sequential waits are semantically identical to ANDed waits."""
    k = 0
    for f in nc.m.functions:
        for b in f.blocks:
            new_list = []
            for inst in b.instructions:
                si = inst.sync_info
                if si is not None and si.on_wait and len(si.on_wait) > 1:
                    waits = list(si.on_wait)
                    for w in waits[:-1]:
                        nop = mybir.InstNoOp(
                            name=f"I-wsplit-{k}",
                            sync_info=mybir.SyncInfo(on_wait=[w], on_update=[]),
                            engine=inst.engine,
                        )
                        k += 1
                        new_list.append(nop)
                    inst.sync_info = mybir.SyncInfo(
                        on_wait=[waits[-1]], on_update=list(si.on_update))
                new_list.append(inst)
            if k:
                b.instructions[:] = new_list


def _sigmoid(x):
    out = np.empty_like(x, dtype=np.float64)
    pos = x >= 0
    out[pos] = 1.0 / (1.0 + np.exp(-x[pos]))
    ex = np.exp(x[~pos])
    out[~pos] = ex / (1.0 + ex)
    return out


def _pack_time(x):
    """(128, T) [p, c*CL+tau] -> (128, T) [p, tau*C + c]."""
    return np.ascontiguousarray(
        x.reshape(128, C, CL).transpose(0, 2, 1).reshape(128, T))


def _host_prologue(dynamics_logits, obs_logits_kc, obs_logits_problem,
                   ability_levels, padded_problem, padded_correct, kc):
    """Build per-core in_maps (numpy only)."""
    dyn_l = np.asarray(dynamics_logits, np.float64)
    obs_kc = np.asarray(obs_logits_kc, np.float64)
    obs_pr = np.asarray(obs_logits_problem, np.float64)
    abil = np.asarray(ability_levels, np.float32)
    prob = np.asarray(padded_problem, np.int64)
    corr = np.asarray(padded_correct, np.int32)
    kc_a = np.asarray(kc, np.int64)

    c0 = obs_kc[kc_a, 0][:, None] + obs_pr[:, 0][prob]    # (S,T)
    c1 = obs_kc[kc_a, 1][:, None] + obs_pr[:, 1][prob]
    w = (2 * corr - 1).astype(np.float64)                 # (S,T) +-1
    u = (w * c0).astype(np.float32)
    v = (-w * c1).astype(np.float32)
    wf = w.astype(np.float32)

    dl = dyn_l[kc_a]                                      # (S,3)
    pL = _sigmoid(dl[:, 0]).astype(np.float32)
    pF = _sigmoid(dl[:, 1]).astype(np.float32)
    pI = _sigmoid(dl[:, 2]).astype(np.float32)

    in_maps = []
    for m in range(NCORES):
        r0, r1 = m * SPC, (m + 1) * SPC
        fa = np.concatenate([
            _pack_time(u[r0:r1]), _pack_time(v[r0:r1]), _pack_time(wf[r0:r1]),
        ], axis=1).astype(np.float16)
        sc = np.zeros((128, NF32), np.float32)
        sc[:, 0] = 1.0 - pL[r0:r1]
        sc[:, 1] = pF[r0:r1]
        sc[:, 2] = pL[r0:r1]
        sc[:, 3] = 1.0 - pF[r0:r1]
        sc[:, 4] = 1.0 - pI[r0:r1]
        sc[:, 5] = pI[r0:r1]
        sc[:, 6:6 + A] = abil[None, :]
        in_maps.append({"F16": fa, "F32": sc})
    return in_maps


def _unshard_ratio(results):
    """Per-core (128, NFLAT) f16 [p, tau*TASKS + c*J + j] -> (AS, T) f32."""
    r = np.empty((A, S, T), np.float32)
    for m in range(NCORES):
        o = np.asarray(results[m]["OUT"]).astype(np.float32)
        o4 = o.reshape(128, CL, C, J).transpose(3, 0, 2, 1)   # (J,128,C,CL)
        r[:, m * SPC:(m + 1) * SPC, :] = o4.reshape(J, SPC, T)
    return r.reshape(AS, T)


def _init_jax_cache(jax):
    try:
        jax.config.update("jax_compilation_cache_dir", "/tmp/jax_comp_cache")
        jax.config.update("jax_persistent_cache_min_compile_time_secs", 0.0)
        jax.config.update("jax_persistent_cache_min_entry_size_bytes", 0)
    except Exception:
        pass


def kernel(dynamics_logits, obs_logits_kc, obs_logits_problem, ability_levels,
           padded_trial_id, padded_problem, padded_correct, kc, ytrue):
    global LAST_EXEC_NS, _NC
    import jax
    import jax.numpy as jnp

    _init_jax_cache(jax)
    cpu = jax.devices("cpu")[0]

    tid = np.asarray(padded_trial_id, np.int32)
    yt = np.asarray(ytrue, np.int32)

    in_maps = _host_prologue(dynamics_logits, obs_logits_kc,
                             obs_logits_problem, ability_levels,
                             padded_problem, padded_correct, kc)

    if _NC is None:
        _NC = _build_nc()

    from concourse.bass_utils import run_bass_kernel_spmd
    import time as _time
    _t0 = _time.perf_counter()
    res = run_bass_kernel_spmd(_NC, in_maps, list(range(NCORES)))
    LAST_EXEC_NS = (_time.perf_counter() - _t0) * 1e9

    rf = _unshard_ratio(res.results)                      # (AS, T) odds ratio
    return _host_epilogue(rf, tid, yt)


def _host_epilogue(rf, tid, yt):
    """Odds ratio (AS,T) -> final logpred (B0, MAX_LEN, 2), mirroring the
    reference scatter + sequential-Bayesian reduction (jax on CPU)."""
    import jax
    import jax.numpy as jnp
    cpu = jax.devices("cpu")[0]

    logp1 = -np.log1p(1.0 / rf)
    logp0 = -np.log1p(rf)

    with jax.default_device(cpu):
        logprob_pred = jnp.asarray(
            np.stack([logp0, logp1], axis=-1))            # (AS,T,2)
        abil_ix = jnp.repeat(jnp.arange(A), S)
        tid_t = jnp.tile(jnp.asarray(tid), (A, 1))
        adj = tid_t + abil_ix[:, None] * (B0 * MAX_LEN)
        adj = jnp.where(tid_t == -1, -1, adj).reshape(-1)
        n_flat = A * B0 * MAX_LEN
        idx = jnp.where(adj > -1, adj, n_flat)
        buf = jnp.zeros((n_flat, 2), dtype=logprob_pred.dtype)
        buf = buf.at[idx].set(logprob_pred.reshape(-1, 2), mode="drop")
        result = jnp.transpose(buf.reshape(A, B0, MAX_LEN, 2), (1, 0, 2, 3))

        ytj = jnp.asarray(yt)
        mask = ytj > -1
        yc = jnp.where(mask, ytj, 0)
        obs_ll = jnp.take_along_axis(
            result, yc[:, None, :, None].astype(jnp.int32), axis=3)[..., 0]
        obs_ll = obs_ll * mask[:, None, :]
        prefix = jnp.cumsum(obs_ll, axis=2) - obs_ll
        from jax.scipy.special import logsumexp
        logw = prefix - logsumexp(prefix, axis=1, keepdims=True)
        logpred = logsumexp(result + logw[..., None], axis=1)
        out = np.asarray(logpred, dtype=np.float32)

    return out


# revision 12
# speedup vs baseline: 7.5163x; 1.7901x over previous
"""Trainium2 Bass kernel for nn_BktModel: BKT HMM forward filter over
A*S=5120 tiled subsequences of length T=1024, followed by scatter into
per-ability timelines and a sequential-Bayesian ability average.

Strategy (v3 — fully on-device, transfer-minimal)
-------------------------------------------------
The spmd call is transfer-bound (axon tunnel ~30-45 MB/s), so everything
heavy runs on device and the wire carries only:

  up:   per core three f16 (128,1024) arrays U = w*c0, V = -w*c1, W = w
        (c0/c1 per-(s,t) observation logits, w = 2*correct-1; padded
        steps get U=V=+50 so both state likelihoods saturate to exactly
        1.0, making them no-ops for the filter and the Bayesian sums),
        plus a (128,144) f32 block: per-subsequence dynamics probs,
        ability levels, and a 128x128 block-triangular matrix.
  down: per core one f16 (128,2048) array: the final logpred
        (students x [log P(incorrect), log P(correct)] x timeline).

Device pipeline per core (128 subsequences on 128 partitions, 5 ability
levels and 32 chunks of 32 steps in the free axis, task = j*32+c):
  1. L0 = sigmoid(U + A_j*W), L1 = sigmoid(V + A_j*W) via scalar engine.
  2. Chunk-parallel basis scan of the linear (unnormalized) HMM filter
     (32 sequential steps over all 160 tasks), periodic rescale.
  3. Chunk endpoint maps prefix-composed by log-doubling -> true chunk
     start states.
  4. Second scan from the true starts emits per step: obs_ll =
     ln(q1/den) (q1 = a0*L0+a1*L1 is the likelihood of the *observed*
     outcome, so no ytrue needed), and the two masked log-predictives
     ln(num1/den), ln(num0/den) (padding detected exactly via q1==den).
  5. Exclusive prefix over each 8192-step student timeline: Hillis-
     Steele within rows + one PE matmul with the shipped triangular
     matrix for the cross-subsequence offsets (8 rows per student live
     on 8 consecutive partitions).
  6. logw = prefix - logsumexp_j(prefix); logpred = logsumexp_j(logp +
     logw) per channel -> f16 out.

Host only packs inputs and reassembles the (128, 8192, 2) output.
"""

import numpy as np

# Problem shape (hardcoded per contract)
B0, K, T, A = 128, 8, 1024, 5
N_KCS, N_PROBLEMS = 50, 1000
MAX_LEN = K * T
S = B0 * K            # 1024 subsequences
AS = A * S
EPS = 1e-12

NCORES = 8
SPC = S // NCORES     # 128 subsequences per core (= partition dim)
STC = B0 // NCORES    # 16 students per core
J = A                 # ability levels, outer task axis
C = 32                # chunks per row
CL = T // C           # 32 steps per chunk
TASKS = J * C         # 160 (ability, chunk) tasks; task = j*C + c
NFLAT = CL * TASKS    # 5120
REN = 16              # rescale period (steps)
PAD_LOGIT = 50.0      # sigmoid(>=48) == 1.0f exactly

NF16 = 3 * T          # U | V | W
NF32 = 16 + 128       # w00 w10 w01 w11 ai0 ai1 A0..A4 pad | tri row
NOUT = 2 * T          # [ch*T + t] per partition

LAST_EXEC_NS = None
_NC = None


def _build_nc():
    import concourse.bass as bass
    import concourse.tile as tile
    from concourse import mybir
    from concourse.bass import MemorySpace
    from contextlib import ExitStack

    f32 = mybir.dt.float32
    f16 = mybir.dt.float16
    OP = mybir.AluOpType
    ACT = mybir.ActivationFunctionType
    nc = bass.Bass()

    dF16 = nc.declare_dram_parameter("F16", [128, NF16], f16, isOutput=False)
    dF32 = nc.declare_dram_parameter("F32", [128, NF32], f32, isOutput=False)
    dOUT = nc.declare_dram_parameter("OUT", [128, NOUT], f16, isOutput=True)

    with ExitStack() as ctx:
        tc = ctx.enter_context(tile.TileContext(nc))
        const = ctx.enter_context(tc.tile_pool(name="const", bufs=1))
        big = ctx.enter_context(tc.tile_pool(name="big", bufs=1))
        work = ctx.enter_context(tc.tile_pool(name="work", bufs=2))
        chain = ctx.enter_context(tc.tile_pool(name="chain", bufs=1))
        ep = ctx.enter_context(tc.tile_pool(name="ep", bufs=1))
        psum = ctx.enter_context(
            tc.tile_pool(name="psum", bufs=1, space=MemorySpace.PSUM))

        V = nc.vector
        SC = nc.scalar

        touch_n = [0]

        def touch(tl):
            tt = const.tile([128, 1], f32, name=f"touch{touch_n[0]}",
                            tag=f"touch{touch_n[0]}")
            touch_n[0] += 1
            V.tensor_copy(tt[:], tl[:, 0:1])

        tsc = const.tile([128, NF32], f32, tag="tsc")
        nc.sync.dma_start(out=tsc[:], in_=dF32[:])
        touch(tsc)
        tf16 = big.tile([128, NF16], f16, tag="tf16")
        nc.sync.dma_start(out=tf16[:], in_=dF16[:])
        touch(tf16)

        w00s = tsc[:, 0:1]
        w10s = tsc[:, 1:2]
        w01s = tsc[:, 2:3]
        w11s = tsc[:, 3:4]
        ai0s = tsc[:, 4:5]
        ai1s = tsc[:, 5:6]
        tri = tsc[:, 16:16 + 128]

        Uf = big.tile([128, T], f32, tag="Uf")
        Vf = big.tile([128, T], f32, tag="Vf")
        Wf = big.tile([128, T], f32, tag="Wf")
        V.tensor_copy(Uf[:], tf16[:, 0:T])
        V.tensor_copy(Vf[:], tf16[:, T:2 * T])
        V.tensor_copy(Wf[:], tf16[:, 2 * T:3 * T])

        # likelihoods per ability level, scan layout [t*TASKS + j*C + c]
        L0 = big.tile([128, NFLAT], f32, tag="L0")
        L1 = big.tile([128, NFLAT], f32, tag="L1")
        for j in range(J):
            Ajs = tsc[:, 6 + j:7 + j]
            argA = work.tile([128, T], f32, tag="argA")
            V.scalar_tensor_tensor(argA[:], Wf[:], Ajs, Uf[:],
                                   op0=OP.mult, op1=OP.add)
            SC.activation(
                L0[:].rearrange("p (t j c) -> p t j c", j=J, c=C)[:, :, j, :],
                argA[:].rearrange("p (t c) -> p t c", c=C), ACT.Sigmoid)
            argB = work.tile([128, T], f32, name="argB", tag="argA")
            V.scalar_tensor_tensor(argB[:], Wf[:], Ajs, Vf[:],
                                   op0=OP.mult, op1=OP.add)
            SC.activation(
                L1[:].rearrange("p (t j c) -> p t j c", j=J, c=C)[:, :, j, :],
                argB[:].rearrange("p (t c) -> p t c", c=C), ACT.Sigmoid)
        # force L0 = L1 = 1.0 exactly at padded steps (U = V = PAD_LOGIT)
        # so padding is detectable on device via q1 == den
        mskT = work.tile([128, T], f32, tag="argA")
        V.tensor_scalar(mskT[:], Uf[:], 40.0, None, op0=OP.is_ge)
        mskb = (mskT[:].rearrange("p (t c) -> p t c", c=C).unsqueeze(2)
                .broadcast_to([128, CL, J, C]))
        for L in (L0, L1):
            Lv = L[:].rearrange("p (t j c) -> p t j c", j=J, c=C)
            V.tensor_max(Lv, Lv, mskb)

        # ---- pass 1: basis scan, endpoints only ----
        sA0 = work.tile([128, TASKS], f32, tag="sA0")
        sA1 = work.tile([128, TASKS], f32, tag="sA1")
        sB0 = work.tile([128, TASKS], f32, tag="sB0")
        sB1 = work.tile([128, TASKS], f32, tag="sB1")
        V.memset(sA0[:], 1.0)
        V.memset(sA1[:], 0.0)
        V.memset(sB0[:], 0.0)
        V.memset(sB1[:], 1.0)
        for t in range(CL):
            l0t = L0[:, t * TASKS:(t + 1) * TASKS]
            l1t = L1[:, t * TASKS:(t + 1) * TASKS]
            nxt = []
            for x0, x1, n0tag, n1tag in ((sA0, sA1, "sA0", "sA1"),
                                         (sB0, sB1, "sB0", "sB1")):
                b0 = work.tile([128, TASKS], f32, tag="b0")
                b1 = work.tile([128, TASKS], f32, tag="b1")
                V.tensor_mul(b0[:], x0[:], l0t)
                V.tensor_mul(b1[:], x1[:], l1t)
                tm0 = work.tile([128, TASKS], f32, tag="tm0")
                V.tensor_scalar_mul(tm0[:], b1[:], w10s)
                n0 = work.tile([128, TASKS], f32, tag=n0tag)
                V.scalar_tensor_tensor(n0[:], b0[:], w00s, tm0[:],
                                       op0=OP.mult, op1=OP.add)
                tm1 = work.tile([128, TASKS], f32, tag="tm1")
                V.tensor_scalar_mul(tm1[:], b1[:], w11s)
                n1 = work.tile([128, TASKS], f32, tag=n1tag)
                V.scalar_tensor_tensor(n1[:], b0[:], w01s, tm1[:],
                                       op0=OP.mult, op1=OP.add)
                nxt.append((n0, n1))
            (sA0, sA1), (sB0, sB1) = nxt
            if (t + 1) % REN == 0:
                sm = work.tile([128, TASKS], f32, tag="rs")
                iv = work.tile([128, TASKS], f32, tag="riv")
                V.tensor_add(sm[:], sA0[:], sA1[:])
                V.reciprocal(iv[:], sm[:])
                for buf in (sA0, sA1, sB0, sB1):
                    V.tensor_mul(buf[:], buf[:], iv[:])

        # ---- chain: prefix-compose chunk maps by log-doubling ----
        # P = [[p00,p01],[p10,p11]] = [[a0A,a0B],[a1A,a1B]] at endpoints
        pcur = [const.tile([128, TASKS], f32, name=f"pc{i}", tag=f"pc{i}")
                for i in range(4)]
        pnx = [const.tile([128, TASKS], f32, name=f"pn{i}", tag=f"pn{i}")
               for i in range(4)]
        V.tensor_copy(pcur[0][:], sA0[:])
        V.tensor_copy(pcur[1][:], sB0[:])
        V.tensor_copy(pcur[2][:], sA1[:])
        V.tensor_copy(pcur[3][:], sB1[:])

        def cv(ap):  # (128, TASKS) -> (128, J, C)
            return ap.rearrange("p (j c) -> p j c", c=C)

        sft = 1
        while sft < C:
            # head c < sft: unchanged
            for i in range(4):
                V.tensor_copy(cv(pnx[i][:])[:, :, 0:sft],
                              cv(pcur[i][:])[:, :, 0:sft])
            # tail: P'[c] = P[c] @ P[c - sft]
            AV = [cv(pcur[i][:])[:, :, sft:C] for i in range(4)]
            BV = [cv(pcur[i][:])[:, :, 0:C - sft] for i in range(4)]
            for i, (ax, ay, bx, by) in enumerate((
                    (AV[0], AV[1], BV[0], BV[2]),   # C00 = A00*B00 + A01*B10
                    (AV[0], AV[1], BV[1], BV[3]),   # C01 = A00*B01 + A01*B11
                    (AV[2], AV[3], BV[0], BV[2]),   # C10
                    (AV[2], AV[3], BV[1], BV[3]))):  # C11
                u = chain.tile([128, TASKS], f32, tag="m0")
                v = chain.tile([128, TASKS], f32, tag="m1")
                V.tensor_mul(cv(u[:])[:, :, 0:C - sft], ax, bx)
                V.tensor_mul(cv(v[:])[:, :, 0:C - sft], ay, by)
                V.tensor_add(cv(pnx[i][:])[:, :, sft:C],
                             cv(u[:])[:, :, 0:C - sft],
                             cv(v[:])[:, :, 0:C - sft])
            # rescale columns by 1/(C00+C10) to keep entries in f32 range
            sa = chain.tile([128, TASKS], f32, tag="s")
            iva = chain.tile([128, TASKS], f32, tag="iv")
            V.tensor_add(sa[:], pnx[0][:], pnx[2][:])
            V.reciprocal(iva[:], sa[:])
            for i in range(4):
                V.tensor_mul(pnx[i][:], pnx[i][:], iva[:])
            pcur, pnx = pnx, pcur
            sft *= 2
        # App[c] = P[c] @ ainit ; a0t[c] = App[c-1] (exclusive), a0t[c=0]=ainit
        a0t = const.tile([128, TASKS], f32, tag="a0t")
        a1t = const.tile([128, TASKS], f32, tag="a1t")
        ap0 = chain.tile([128, TASKS], f32, tag="m2")
        ap1 = chain.tile([128, TASKS], f32, tag="m3")
        v0 = chain.tile([128, TASKS], f32, tag="m0")
        V.tensor_scalar_mul(v0[:], pcur[1][:], ai1s)
        V.scalar_tensor_tensor(ap0[:], pcur[0][:], ai0s, v0[:],
                               op0=OP.mult, op1=OP.add)
        v1 = chain.tile([128, TASKS], f32, tag="m1")
        V.tensor_scalar_mul(v1[:], pcur[3][:], ai1s)
        V.scalar_tensor_tensor(ap1[:], pcur[2][:], ai0s, v1[:],
                               op0=OP.mult, op1=OP.add)
        for at, ais in ((a0t, ai0s), (a1t, ai1s)):
            V.memset(cv(at[:])[:, :, 0:1], 1.0)
            V.tensor_scalar_mul(cv(at[:])[:, :, 0:1], cv(at[:])[:, :, 0:1], ais)
        V.tensor_copy(cv(a0t[:])[:, :, 1:C], cv(ap0[:])[:, :, 0:C - 1])
        V.tensor_copy(cv(a1t[:])[:, :, 1:C], cv(ap1[:])[:, :, 0:C - 1])

        # ---- pass 2: true scan, emit obs_ll + masked log-predictives ----
        # timeline layout [p, j*T + c*CL + tau]
        OLL = big.tile([128, J * T], f32, tag="OLL")
        LP0 = big.tile([128, J * T], f32, tag="LP0")
        LP1 = big.tile([128, J * T], f32, tag="LP1")

        y0, y1 = a0t, a1t
        for t in range(CL):
            l0t = L0[:, t * TASKS:(t + 1) * TASKS]
            l1t = L1[:, t * TASKS:(t + 1) * TASKS]
            b0 = work.tile([128, TASKS], f32, tag="b0")
            b1 = work.tile([128, TASKS], f32, tag="b1")
            V.tensor_mul(b0[:], y0[:], l0t)
            V.tensor_mul(b1[:], y1[:], l1t)
            q1 = work.tile([128, TASKS], f32, tag="q1")
            den = work.tile([128, TASKS], f32, tag="den")
            V.tensor_add(q1[:], b0[:], b1[:])
            V.tensor_add(den[:], y0[:], y1[:])
            lnq = work.tile([128, TASKS], f32, tag="lnq")
            lnd = work.tile([128, TASKS], f32, tag="lnd")
            SC.activation(lnq[:], q1[:], ACT.Ln)
            SC.activation(lnd[:], den[:], ACT.Ln)
            # obs_ll -> OLL[:, j*T + c*CL + t]
            dst = OLL[:].rearrange("p (j c tt) -> p j c tt", c=C, tt=CL)
            V.tensor_sub(dst[:, :, :, t], cv(lnq[:]), cv(lnd[:]))
            # num1 = hd - w*(hd - q1), hd = den/2  (== 0.5(1-w)den + w*q1)
            wt = Wf[:, t * C:(t + 1) * C].unsqueeze(1).broadcast_to([128, J, C])
            hd = work.tile([128, TASKS], f32, tag="hd")
            V.tensor_scalar(hd[:], den[:], 0.5, None, op0=OP.mult)
            qw = work.tile([128, TASKS], f32, tag="qw")
            V.tensor_sub(qw[:], hd[:], q1[:])
            V.tensor_mul(cv(qw[:]), cv(qw[:]), wt)
            num1 = work.tile([128, TASKS], f32, tag="num1")
            V.tensor_sub(num1[:], hd[:], qw[:])
            num0 = work.tile([128, TASKS], f32, tag="num0")
            V.tensor_sub(num0[:], den[:], num1[:])
            # padding mask: q1 == den exactly iff L0 == L1 == 1 (padded)
            msk = work.tile([128, TASKS], f32, tag="msk")
            V.tensor_tensor(msk[:], q1[:], den[:], op=OP.is_equal)
            mm = work.tile([128, TASKS], f32, tag="mm")
            V.tensor_scalar(mm[:], msk[:], -1.0, 1.0, op0=OP.mult, op1=OP.add)
            V.tensor_scalar_max(num1[:], num1[:], 1e-38)
            V.tensor_scalar_max(num0[:], num0[:], 1e-38)
            ln1 = work.tile([128, TASKS], f32, tag="ln1")
            ln0 = work.tile([128, TASKS], f32, tag="ln0")
            SC.activation(ln1[:], num1[:], ACT.Ln)
            SC.activation(ln0[:], num0[:], ACT.Ln)
            lp1 = work.tile([128, TASKS], f32, tag="lp1")
            lp0 = work.tile([128, TASKS], f32, tag="lp0")
            V.tensor_sub(lp1[:], ln1[:], lnd[:])
            V.tensor_sub(lp0[:], ln0[:], lnd[:])
            d1 = LP1[:].rearrange("p (j c tt) -> p j c tt", c=C, tt=CL)
            d0 = LP0[:].rearrange("p (j c tt) -> p j c tt", c=C, tt=CL)
            V.tensor_mul(d1[:, :, :, t], cv(lp1[:]), cv(mm[:]))
            V.tensor_mul(d0[:, :, :, t], cv(lp0[:]), cv(mm[:]))
            # evolve
            tm0 = work.tile([128, TASKS], f32, tag="tm0")
            V.tensor_scalar_mul(tm0[:], b1[:], w10s)
            ny0 = work.tile([128, TASKS], f32, tag="y0")
            V.scalar_tensor_tensor(ny0[:], b0[:], w00s, tm0[:],
                                   op0=OP.mult, op1=OP.add)
            tm1 = work.tile([128, TASKS], f32, tag="tm1")
            V.tensor_scalar_mul(tm1[:], b1[:], w11s)
            ny1 = work.tile([128, TASKS], f32, tag="y1")
            V.scalar_tensor_tensor(ny1[:], b0[:], w01s, tm1[:],
                                   op0=OP.mult, op1=OP.add)
            y0, y1 = ny0, ny1
            if (t + 1) % REN == 0 and t + 1 < CL:
                iv = work.tile([128, TASKS], f32, tag="riv")
                V.reciprocal(iv[:], den[:])
                V.tensor_mul(y0[:], y0[:], iv[:])
                V.tensor_mul(y1[:], y1[:], iv[:])

        # ---- exclusive prefix over the 8192-step student timelines ----
        def jt(ap):  # (128, J*T) -> (128, J, T)
            return ap.rearrange("p (j t) -> p j t", j=J)

        PRE = big.tile([128, J * T], f32, tag="PRE")
        R0 = ep.tile([128, J], f32, tag="R0")
        V.tensor_copy(R0[:].unsqueeze(2), jt(OLL[:])[:, :, T - 1:T])
        V.memset(jt(PRE[:])[:, :, 0:1], 0.0)
        V.tensor_copy(jt(PRE[:])[:, :, 1:T], jt(OLL[:])[:, :, 0:T - 1])
        cur, oth = PRE, OLL
        s = 1
        while s < T:
            V.tensor_copy(jt(oth[:])[:, :, 0:s], jt(cur[:])[:, :, 0:s])
            V.tensor_add(jt(oth[:])[:, :, s:T], jt(cur[:])[:, :, s:T],
                         jt(cur[:])[:, :, 0:T - s])
            cur, oth = oth, cur
            s *= 2
        # row totals -> per-partition offsets via PE (tri is strict
        # block-lower-triangular within each student's 8 partitions)
        R = ep.tile([128, J], f32, tag="R")
        V.tensor_add(R[:].unsqueeze(2), jt(cur[:])[:, :, T - 1:T],
                     R0[:].unsqueeze(2))
        offp = psum.tile([128, J], f32, tag="offp")
        nc.tensor.matmul(offp[:], tri, R[:], start=True, stop=True)
        offs = ep.tile([128, J], f32, tag="offs")
        V.tensor_copy(offs[:], offp[:])
        V.tensor_add(jt(cur[:]), jt(cur[:]),
                     offs[:].unsqueeze(2).broadcast_to([128, J, T]))

        # ---- logw = pre - logsumexp_j(pre); logpred = lse_j(logp+logw) ----
        prej = [cur[:, j * T:(j + 1) * T] for j in range(J)]
        othj = [oth[:, j * T:(j + 1) * T] for j in range(J)]
        mx = ep.tile([128, T], f32, tag="mx")
        V.tensor_max(mx[:], prej[0], prej[1])
        for j in range(2, J):
            V.tensor_max(mx[:], mx[:], prej[j])
        for j in range(J):
            V.tensor_sub(othj[j], prej[j], mx[:])
            SC.activation(othj[j], othj[j], ACT.Exp)
        sm = ep.tile([128, T], f32, tag="sm")
        V.tensor_add(sm[:], othj[0], othj[1])
        for j in range(2, J):
            V.tensor_add(sm[:], sm[:], othj[j])
        SC.activation(sm[:], sm[:], ACT.Ln)
        V.tensor_add(mx[:], sm[:], mx[:])            # mx now holds lse
        for j in range(J):
            V.tensor_sub(prej[j], prej[j], mx[:])    # cur now holds logw

        OUTE = big.tile([128, NOUT], f16, tag="OUTE")
        for ch, LP in ((0, LP0), (1, LP1)):
            for j in range(J):
                V.tensor_add(othj[j], LP[:, j * T:(j + 1) * T], prej[j])
            V.tensor_max(mx[:], othj[0], othj[1])
            for j in range(2, J):
                V.tensor_max(mx[:], mx[:], othj[j])
            for j in range(J):
                V.tensor_sub(othj[j], othj[j], mx[:])
                SC.activation(othj[j], othj[j], ACT.Exp)
            V.tensor_add(sm[:], othj[0], othj[1])
            for j in range(2, J):
                V.tensor_add(sm[:], sm[:], othj[j])
            SC.activation(sm[:], sm[:], ACT.Ln)
            V.tensor_add(OUTE[:, ch * T:(ch + 1) * T], sm[:], mx[:])

        nc.sync.dma_start(out=dOUT[:], in_=OUTE[:])

    _split_multi_waits(nc, mybir)
    return nc


def _split_multi_waits(nc, mybir):
    """This neuronx-cc codegen allows only one sync-wait slot per
    instruction; hoist all but the last wait of any multi-wait instruction
    onto single-wait NoOps inserted just before it (same engine, same
    block) - sequential waits are semantically identical to ANDed waits."""
    k = 0
    for f in nc.m.functions:
        for b in f.blocks:
            new_list = []
            for inst in b.instructions:
                si = inst.sync_info
                if si is not None and si.on_wait and len(si.on_wait) > 1:
                    waits = list(si.on_wait)
                    for w in waits[:-1]:
                        nop = mybir.InstNoOp(
                            name=f"I-wsplit-{k}",
                            sync_info=mybir.SyncInfo(on_wait=[w], on_update=[]),
                            engine=inst.engine,
                        )
                        k += 1
                        new_list.append(nop)
                    inst.sync_info = mybir.SyncInfo(
                        on_wait=[waits[-1]], on_update=list(si.on_update))
                new_list.append(inst)
            if k:
                b.instructions[:] = new_list


def _sigmoid(x):
    out = np.empty_like(x, dtype=np.float64)
    pos = x >= 0
    out[pos] = 1.0 / (1.0 + np.exp(-x[pos]))
    ex = np.exp(x[~pos])
    out[~pos] = ex / (1.0 + ex)
    return out


def _pack_time(x):
    """(128, T) [p, c*CL+tau] -> (128, T) [p, tau*C + c]."""
    return np.ascontiguousarray(
        x.reshape(128, C, CL).transpose(0, 2, 1).reshape(128, T))


def _tri_matrix():
    """tri[p, p'] = 1 iff same student (p//8==p'//8) and p%8 < p'%8."""
    p = np.arange(128)
    return ((p[:, None] // K == p[None, :] // K)
            & (p[:, None] % K < p[None, :] % K)).astype(np.float32)


def _host_prologue(dynamics_logits, obs_logits_kc, obs_logits_problem,
                   ability_levels, padded_trial_id, padded_problem,
                   padded_correct, kc):
    """Build per-core in_maps (numpy only)."""
    dyn_l = np.asarray(dynamics_logits, np.float64)
    obs_kc = np.asarray(obs_logits_kc, np.float64)
    obs_pr = np.asarray(obs_logits_problem, np.float64)
    abil = np.asarray(ability_levels, np.float32)
    tid = np.asarray(padded_trial_id, np.int32)
    prob = np.asarray(padded_problem, np.int64)
    corr = np.asarray(padded_correct, np.int32)
    kc_a = np.asarray(kc, np.int64)

    c0 = obs_kc[kc_a, 0][:, None] + obs_pr[:, 0][prob]    # (S,T)
    c1 = obs_kc[kc_a, 1][:, None] + obs_pr[:, 1][prob]
    w = (2 * corr - 1).astype(np.float64)                 # (S,T) +-1
    valid = tid != -1
    u = np.where(valid, w * c0, PAD_LOGIT).astype(np.float32)
    v = np.where(valid, -w * c1, PAD_LOGIT).astype(np.float32)
    wf = w.astype(np.float32)

    dl = dyn_l[kc_a]                                      # (S,3)
    pL = _sigmoid(dl[:, 0]).astype(np.float32)
    pF = _sigmoid(dl[:, 1]).astype(np.float32)
    pI = _sigmoid(dl[:, 2]).astype(np.float32)
    tri = _tri_matrix()

    in_maps = []
    for m in range(NCORES):
        r0, r1 = m * SPC, (m + 1) * SPC
        fa = np.concatenate([
            _pack_time(u[r0:r1]), _pack_time(v[r0:r1]), _pack_time(wf[r0:r1]),
        ], axis=1).astype(np.float16)
        sc = np.zeros((128, NF32), np.float32)
        sc[:, 0] = 1.0 - pL[r0:r1]
        sc[:, 1] = pF[r0:r1]
        sc[:, 2] = pL[r0:r1]
        sc[:, 3] = 1.0 - pF[r0:r1]
        sc[:, 4] = 1.0 - pI[r0:r1]
        sc[:, 5] = pI[r0:r1]
        sc[:, 6:6 + A] = abil[None, :]
        sc[:, 16:144] = tri
        in_maps.append({"F16": fa, "F32": sc})
    return in_maps


def _unshard_logpred(results):
    """Per-core (128, NOUT) f16 [p, ch*T + t] -> (B0, MAX_LEN, 2) f32."""
    out = np.empty((B0, MAX_LEN, 2), np.float32)
    for m in range(NCORES):
        o = np.asarray(results[m]["OUT"]).astype(np.float32)  # (128, 2T)
        o4 = o.reshape(STC, K, 2, T).transpose(0, 1, 3, 2)    # (16,K,T,2)
        out[m * STC:(m + 1) * STC] = o4.reshape(STC, MAX_LEN, 2)
    return out


def _init_jax_cache(jax):
    try:
        jax.config.update("jax_compilation_cache_dir", "/tmp/jax_comp_cache")
        jax.config.update("jax_persistent_cache_min_compile_time_secs", 0.0)
        jax.config.update("jax_persistent_cache_min_entry_size_bytes", 0)
    except Exception:
        pass


def kernel(dynamics_logits, obs_logits_kc, obs_logits_problem, ability_levels,
           padded_trial_id, padded_problem, padded_correct, kc, ytrue):
    global LAST_EXEC_NS, _NC
    import jax
    _init_jax_cache(jax)

    in_maps = _host_prologue(dynamics_logits, obs_logits_kc,
                             obs_logits_problem, ability_levels,
                             padded_trial_id, padded_problem,
                             padded_correct, kc)

    if _NC is None:
        _NC = _build_nc()

    from concourse.bass_utils import run_bass_kernel_spmd
    import time as _time
    _t0 = _time.perf_counter()
    res = run_bass_kernel_spmd(_NC, in_maps, list(range(NCORES)))
    LAST_EXEC_NS = (_time.perf_counter() - _t0) * 1e9

    return _unshard_logpred(res.results)
